# revision 13
# baseline (speedup 1.0000x reference)
"""Trainium2 Bass kernel for nn_Encoder_With_Codebook (vq_codebook).

Strategy: data-parallel over the 16384-cell batch across 8 NeuronCores, in a
"transposed world" layout (features on SBUF partitions, cells on the free dim)
so every linear layer chains without transposes and BatchNorm is per-partition.
BN batch statistics are all-reduced across cores in-kernel (5 tiny AllReduces).
Big matmuls run as float32r (full PE rate, ~1.5e-4 rel err). The per-codebook
softmax numerator exp(logits) is computed on device and emitted raw; the cheap
row-normalization, argmax/codebook-loss scalar, and final transposes are
finished on the host from device outputs.
"""
import sys

sys.path.insert(0, "/opt/trn_rl_repo")

import numpy as np

NCORES = 8
N = 16384
NLOC = N // NCORES          # 2048 rows per core
D_IN = 2000
D_PAD = 2048                # padded contraction for enc0
HID0, HID1, EMB = 1024, 512, 512
NCB, K, CBD = 4, 1024, 64
ZD = 128
H2 = 2 * ZD                 # 256
FF = 4 * H2                 # 1024
BETA, SCALER = 0.25, 1.0
EPS = 1e-5
LSHIFT = 40.0               # logit shift folded into ee to avoid fp32 overflow

_CACHE = {}


def _build():
    import concourse.bacc as bacc
    import concourse.tile as tile
    import concourse.mybir as mybir
    from concourse.bass_interp import get_hw_module

    dt = mybir.dt
    f32, f32r = dt.float32, dt.float32r
    AF = mybir.ActivationFunctionType
    ALU = mybir.AluOpType
    AX = mybir.AxisListType

    nc = bacc.Bacc("TRN2", target_bir_lowering=False, debug=False,
                   enable_asserts=False, num_devices=NCORES)

    def din(name, shape):
        return nc.dram_tensor(name, list(shape), f32, kind="ExternalInput").ap()

    def dout(name, shape):
        return nc.dram_tensor(name, list(shape), f32, kind="ExternalOutput").ap()

    # ---- inputs (per-core shards / replicated weights) ----
    xin_d = din("xin", (8, 16, 128, 256))         # exp.T padded, n-sliced
    eg_d = din("eg", (NCB, K, NLOC))              # exp(gumbels) transposed
    w0_d = din("w0", (16, 128, HID0))             # enc0_W padded (2048,1024)
    b0_d = din("b0v", (128, 8))
    g0_d = din("g0v", (128, 8))
    be0_d = din("be0v", (128, 8))
    w1_d = din("w1", (8, 128, HID1))
    b1_d = din("b1v", (128, 4))
    g1_d = din("g1v", (128, 4))
    be1_d = din("be1v", (128, 4))
    w2_d = din("w2", (4, 128, EMB))
    b2_d = din("b2v", (128, 4))
    wcb_d = din("wcb", (4, 128, NCB * CBD))       # concat cb W (512,256)
    bcb_d = din("bcbv", (128, 2))
    gcb_d = din("gcbv", (128, 2))
    becb_d = din("becbv", (128, 2))
    e2t_d = din("e2t", (128, 2, K))               # 2*E.T cb-interleaved
    ez_d = din("ez", (NCB, 8, 128, CBD + 1))      # [E | ones] k-chunked
    eet_d = din("eet", (128, NCB, 8))             # ||e||^2 + LSHIFT, k-parts
    wlin_d = din("wlin", (2, 128, H2))
    blin_d = din("blinv", (128, 2))
    gb1g_d = din("gb1g", (128, 2))
    gb1b_d = din("gb1b", (128, 2))
    wf1_d = din("wfc1", (2, 128, FF))
    bf1_d = din("bfc1v", (128, 8))
    wf2_d = din("wfc2", (8, 128, H2))
    bf2_d = din("bfc2v", (128, 2))
    gb2g_d = din("gb2g", (128, 2))
    gb2b_d = din("gb2b", (128, 2))
    wm_d = din("wmean", (2, 128, ZD))
    bm_d = din("bmeanv", (128, 1))
    wv_d = din("wvar", (2, 128, ZD))
    bv_d = din("bvarv", (128, 1))

    # ---- outputs ----
    expt_o = dout("expt_out", (NCB, K, NLOC))
    zt_o = dout("zt_out", (NCB, CBD, NLOC))
    zm_o = dout("zmeant_out", (ZD, NLOC))
    vr_o = dout("vart_out", (ZD, NLOC))

    RG = [list(range(NCORES))]
    INVN = 1.0 / float(N)

    with tile.TileContext(nc) as tc:
        open_pools = []

        def popen(**kw):
            cm = tc.tile_pool(**kw)
            pool = cm.__enter__()
            open_pools.append(cm)
            return cm, pool

        def pclose(cm):
            open_pools.remove(cm)
            cm.__exit__(None, None, None)

        gcm, glob = popen(name="glob", bufs=1, side="left")
        dcm, dpool = popen(name="dram", bufs=1, space="DRAM")

        # ---------- small helpers ----------
        def allreduce(idx, st_sbuf, width):
            ari = dpool.tile([128, width], f32, tag=f"ari{idx}",
                             name=f"ari{idx}")
            aro = dpool.tile([128, width], f32, tag=f"aro{idx}",
                             addr_space="Shared", name=f"aro{idx}")
            nc.sync.dma_start(ari[:], st_sbuf)
            nc.gpsimd.collective_compute(
                "AllReduce", ALU.add, replica_groups=RG,
                ins=[ari[:]], outs=[aro[:]])
            gst = glob.tile([128, width], f32, tag=f"gst{idx}",
                            name=f"gst{idx}")
            nc.sync.dma_start(gst[:], aro[:])
            return gst

        def bn_coeffs(idx, gst, w, g_ap, be_ap):
            mu = glob.tile([128, w], f32, tag=f"mu{idx}", name=f"mu{idx}")
            va = glob.tile([128, w], f32, tag=f"va{idx}", name=f"va{idx}")
            sd = glob.tile([128, w], f32, tag=f"sd{idx}", name=f"sd{idx}")
            aa = glob.tile([128, w], f32, tag=f"aa{idx}", name=f"aa{idx}")
            cc = glob.tile([128, w], f32, tag=f"cc{idx}", name=f"cc{idx}")
            nc.vector.tensor_scalar_mul(mu[:], gst[:, 0:w], INVN)
            nc.vector.tensor_scalar_mul(va[:], gst[:, w:2 * w], INVN)
            nc.vector.tensor_mul(sd[:], mu[:], mu[:])
            nc.vector.tensor_sub(va[:], va[:], sd[:])
            nc.scalar.activation(sd[:], va[:], AF.Sqrt, bias=epsv[:, 0:1])
            nc.vector.reciprocal(aa[:], sd[:])
            nc.vector.tensor_mul(aa[:], aa[:], g_ap)
            nc.vector.tensor_mul(cc[:], mu[:], aa[:])
            nc.vector.tensor_sub(cc[:], be_ap, cc[:])
            return aa, cc

        def load_small(ap_d, shape, tag):
            t = glob.tile(list(shape), f32, tag=tag, name=tag)
            nc.sync.dma_start(t[:], ap_d[:])
            return t

        def load_wr(pool, dram3, kchunks, mwidth, tag):
            """Load (kchunks,128,mwidth) weights, convert to f32r chunkwise."""
            wr = pool.tile([128, kchunks, mwidth], f32r, tag=tag, name=tag)
            for k in range(kchunks):
                ws = glob.tile([128, 1024], f32, tag="wstage", name="ws",
                               bufs=2)
                nc.sync.dma_start(ws[:, 0:mwidth], dram3[k])
                nc.vector.tensor_copy(wr[:, k, :], ws[:, 0:mwidth])
            return wr

        def sumsq_pass(src_ap, m_count, acc_tile, acc_off):
            """One x^2+accum STT pass per m chunk into acc_tile cols."""
            scm, scp = popen(name="scrp", bufs=2, side="left")
            for m in range(m_count):
                sc = scp.tile([128, NLOC], f32, tag="sc", name="sc")
                nc.vector.scalar_tensor_tensor(
                    sc[:], src_ap[:, m, :], 0.0, src_ap[:, m, :],
                    ALU.add, ALU.mult,
                    accum_out=acc_tile[:, acc_off + m:acc_off + m + 1])
            pclose(scm)

        epsv = glob.tile([128, 1], f32, tag="epsv", name="epsv")
        nc.vector.memset(epsv[:], EPS)

        b0v = load_small(b0_d, (128, 8), "b0v")
        g0v = load_small(g0_d, (128, 8), "g0v")
        be0v = load_small(be0_d, (128, 8), "be0v")
        b1v = load_small(b1_d, (128, 4), "b1v")
        g1v = load_small(g1_d, (128, 4), "g1v")
        be1v = load_small(be1_d, (128, 4), "be1v")
        b2v = load_small(b2_d, (128, 4), "b2v")
        bcbv = load_small(bcb_d, (128, 2), "bcbv")
        gcbv = load_small(gcb_d, (128, 2), "gcbv")
        becbv = load_small(becb_d, (128, 2), "becbv")
        blinv = load_small(blin_d, (128, 2), "blinv")
        gb1g = load_small(gb1g_d, (128, 2), "gb1g")
        gb1b = load_small(gb1b_d, (128, 2), "gb1b")
        bf1v = load_small(bf1_d, (128, 8), "bf1v")
        bf2v = load_small(bf2_d, (128, 2), "bf2v")
        gb2g = load_small(gb2g_d, (128, 2), "gb2g")
        gb2b = load_small(gb2b_d, (128, 2), "gb2b")
        bmv = load_small(bm_d, (128, 1), "bmv")
        bvv = load_small(bv_d, (128, 1), "bvv")
        eets = load_small(eet_d, (128, NCB * 8), "eets")

        # =========================================================
        # enc0: x0 = exp @ W0 + b0   (k=16 chunks, m=8, n=8 x 256)
        # =========================================================
        p1cm, p1 = popen(name="p1", bufs=1, side="left")     # w0r + xin/xr staging
        p2cm, p2 = popen(name="p2", bufs=1, side="right")     # x0_pre
        w0r = load_wr(p1, w0_d, 16, HID0, "w0r")
        x0_pre = p2.tile([128, 8, NLOC], f32, tag="x0pre", name="x0_pre")
        s0acc = glob.tile([128, 8, 8], f32, tag="s0acc", name="s0acc")
        with tc.tile_pool(name="ps0", bufs=4, space="PSUM") as ps0:
            for n in range(8):
                xr = p1.tile([128, 16, 256], f32r, tag="xr", name="xr",
                             bufs=2)
                for kq in range(4):
                    xin_s = p1.tile([128, 4, 256], f32, tag="xins",
                                    name="xin_s", bufs=2)
                    nc.sync.dma_start(xin_s[:], xin_d[n, 4 * kq:4 * kq + 4]
                                      .rearrange("k p w -> p k w"))
                    nc.vector.tensor_copy(xr[:, 4 * kq:4 * kq + 4, :],
                                          xin_s[:])
                for m in range(8):
                    ps = ps0.tile([128, 256], f32, tag="mm", name="ps")
                    for k in range(16):
                        nc.tensor.matmul(
                            ps[:], w0r[:, k, 128 * m:128 * (m + 1)],
                            xr[:, k, :], start=(k == 0), stop=(k == 15))
                    nc.scalar.activation(
                        x0_pre[:, m, 256 * n:256 * (n + 1)], ps[:],
                        AF.Identity, bias=b0v[:, m:m + 1],
                        accum_out=s0acc[:, m, n:n + 1])
        pclose(p1cm)
        st0 = glob.tile([128, 16], f32, tag="st0", name="st0")
        nc.vector.tensor_reduce(st0[:, 0:8], s0acc[:], axis=AX.X, op=ALU.add)
        sumsq_pass(x0_pre, 8, st0, 8)
        gst0 = allreduce(0, st0[:], 16)
        a0, c0 = bn_coeffs(0, gst0, 8, g0v[:], be0v[:])
        px0cm, px0 = popen(name="px0", bufs=1, side="left")
        x0r = px0.tile([128, 8, NLOC], f32r, tag="x0r", name="x0r")
        for m in range(8):
            nc.scalar.activation(x0r[:, m, :], x0_pre[:, m, :], AF.Relu,
                                 bias=c0[:, m:m + 1], scale=a0[:, m:m + 1])
        pclose(p2cm)

        # =========================================================
        # enc1: x1 = relu(bn(x0)) @ W1 + b1   (k=8, m=4, n=4 x 512)
        # =========================================================
        p3cm, p3 = popen(name="p3", bufs=1, side="right")
        w1r = load_wr(p3, w1_d, 8, HID1, "w1r")
        x1_pre = p3.tile([128, 4, NLOC], f32, tag="x1pre", name="x1_pre")
        s1acc = glob.tile([128, 4, 4], f32, tag="s1acc", name="s1acc")
        with tc.tile_pool(name="ps1", bufs=4, space="PSUM") as ps1:
            for n in range(4):
                for m in range(4):
                    ps = ps1.tile([128, 512], f32, tag="mm", name="ps")
                    for k in range(8):
                        nc.tensor.matmul(
                            ps[:], w1r[:, k, 128 * m:128 * (m + 1)],
                            x0r[:, k, 512 * n:512 * (n + 1)],
                            start=(k == 0), stop=(k == 7))
                    nc.scalar.activation(
                        x1_pre[:, m, 512 * n:512 * (n + 1)], ps[:],
                        AF.Identity, bias=b1v[:, m:m + 1],
                        accum_out=s1acc[:, m, n:n + 1])
        pclose(px0cm)
        st1 = glob.tile([128, 8], f32, tag="st1", name="st1")
        nc.vector.tensor_reduce(st1[:, 0:4], s1acc[:], axis=AX.X, op=ALU.add)
        sumsq_pass(x1_pre, 4, st1, 4)
        gst1 = allreduce(1, st1[:], 8)
        a1, c1 = bn_coeffs(1, gst1, 4, g1v[:], be1v[:])
        px1cm, px1 = popen(name="px1", bufs=1, side="left")
        x1r = px1.tile([128, 4, NLOC], f32r, tag="x1r", name="x1r")
        for m in range(4):
            nc.scalar.activation(x1r[:, m, :], x1_pre[:, m, :], AF.Relu,
                                 bias=c1[:, m:m + 1], scale=a1[:, m:m + 1])
        pclose(p3cm)

        # =========================================================
        # enc2: xe = relu(bn(x1)) @ W2 + b2 ; cbproj: y = xe@Wcat + bcat
        # =========================================================
        p4cm, p4 = popen(name="p4", bufs=1, side="right")
        w2r = load_wr(p4, w2_d, 4, EMB, "w2r")
        xe = p4.tile([128, 4, NLOC], f32r, tag="xe", name="xe")
        with tc.tile_pool(name="ps2", bufs=4, space="PSUM") as ps2:
            for n in range(4):
                for m in range(4):
                    ps = ps2.tile([128, 512], f32, tag="mm", name="ps")
                    for k in range(4):
                        nc.tensor.matmul(
                            ps[:], w2r[:, k, 128 * m:128 * (m + 1)],
                            x1r[:, k, 512 * n:512 * (n + 1)],
                            start=(k == 0), stop=(k == 3))
                    nc.scalar.activation(
                        xe[:, m, 512 * n:512 * (n + 1)], ps[:],
                        AF.Identity, bias=b2v[:, m:m + 1])
        pclose(px1cm)
        pycm, py = popen(name="py", bufs=1, side="left")
        y_pre = py.tile([128, 2, NLOC], f32r, tag="ypre", name="y_pre")
        sYacc = glob.tile([128, 2, 4], f32, tag="syacc", name="sYacc")
        wcr = load_wr(p4, wcb_d, 4, NCB * CBD, "wcr")
        with tc.tile_pool(name="ps3", bufs=4, space="PSUM") as ps3:
            for n in range(4):
                for m in range(2):
                    ps = ps3.tile([128, 512], f32, tag="mm", name="ps")
                    for k in range(4):
                        nc.tensor.matmul(
                            ps[:], wcr[:, k, 128 * m:128 * (m + 1)],
                            xe[:, k, 512 * n:512 * (n + 1)],
                            start=(k == 0), stop=(k == 3))
                    nc.scalar.activation(
                        y_pre[:, m, 512 * n:512 * (n + 1)], ps[:],
                        AF.Identity, bias=bcbv[:, m:m + 1],
                        accum_out=sYacc[:, m, n:n + 1])
        pclose(p4cm)
        stY = glob.tile([128, 4], f32, tag="stY", name="stY")
        nc.vector.tensor_reduce(stY[:, 0:2], sYacc[:], axis=AX.X, op=ALU.add)
        sumsq_pass(y_pre, 2, stY, 2)
        gstY = allreduce(2, stY[:], 4)
        aY, cY = bn_coeffs(2, gstY, 2, gcbv[:], becbv[:])

        # ---- codebook constants: E2a = a*(2E.T) f32r, biask, Ez f32r ----
        pccm, pcc = popen(name="pcc", bufs=1, side="left")    # e2a, ezr (persist cb phase)
        e2a = pcc.tile([128, 2, K], f32r, tag="e2a", name="e2a")
        ezr = pcc.tile([128, NCB, 8, CBD + 1], f32r, tag="ezr", name="ezr")
        biask = glob.tile([128, NCB, 8], f32, tag="biask", name="biask")
        c2t = glob.tile([128, 2], f32, tag="c2t", name="c2t")
        ones64 = glob.tile([1, 64], f32, tag="ones64", name="ones64")
        nc.vector.memset(ones64[:], 1.0)
        ones64r = glob.tile([1, 64], f32r, tag="ones64r", name="ones64r")
        nc.vector.tensor_copy(ones64r[:], ones64[:])
        pstcm, pst = popen(name="pst", bufs=1, side="left")   # f32 staging, closed early
        e2s = pst.tile([128, 2, K], f32, tag="e2s", name="e2s")
        nc.sync.dma_start(e2s[:], e2t_d[:])
        ezs = pst.tile([128, NCB, 8, CBD + 1], f32, tag="ezs", name="ezs")
        nc.sync.dma_start(ezs[:], ez_d.rearrange("i c p w -> p i c w"))
        nc.vector.tensor_copy(ezr[:], ezs[:])
        # e2s already carries the 2x factor, so bias = (2E).c - ee uses c as-is
        nc.vector.tensor_copy(c2t[:], cY[:])
        for j in range(2):
            nc.vector.tensor_scalar(e2a[:, j, :], e2s[:, j, :],
                                    aY[:, j:j + 1], None, ALU.mult)
        with tc.tile_pool(name="psb0", bufs=2, space="PSUM") as psb0:
            for i in range(NCB):
                pb_lo = 64 * (i % 2)
                for c in range(8):
                    psb = psb0.tile([128, 1], f32, tag="psb", name="psb")
                    nc.tensor.matmul(
                        psb[:],
                        e2s[pb_lo:pb_lo + 64, i // 2, 128 * c:128 * (c + 1)],
                        c2t[pb_lo:pb_lo + 64, i // 2:i // 2 + 1],
                        start=True, stop=True)
                    nc.vector.tensor_sub(
                        biask[:, i, c:c + 1], psb[:],
                        eets[:, 8 * i + c:8 * i + c + 1])
        pclose(pstcm)

        # =========================================================
        # codebook phase: logits -> exp -> (x eg) -> z, per (i, rt)
        # =========================================================
        pzcm, pzt = popen(name="pzt", bufs=1, side="right")
        ztc = pzt.tile([128, 2, NLOC], f32, tag="ztc", name="ztc")
        pcbcm, pcb = popen(name="pcb", bufs=1, side="left")
        with tc.tile_pool(name="psl", bufs=3, space="PSUM") as psl, \
             tc.tile_pool(name="psz", bufs=2, space="PSUM") as psz, \
             tc.tile_pool(name="psq", bufs=2, space="PSUM") as psq:
            for i in range(NCB):
                pb_lo = 64 * (i % 2)
                for rt in range(4):
                    r0, r1 = 512 * rt, 512 * (rt + 1)
                    y_sl = y_pre[pb_lo:pb_lo + 64, i // 2, r0:r1]
                    pz = psz.tile([65, 512], f32, tag="pz", name="pz")
                    for c in range(8):
                        pl = psl.tile([128, 512], f32, tag="pl", name="pl")
                        nc.tensor.matmul(
                            pl[:],
                            e2a[pb_lo:pb_lo + 64, i // 2,
                                128 * c:128 * (c + 1)],
                            y_sl, start=True, stop=True)
                        ext = pcb.tile([128, 512], f32, tag="ext",
                                       name="ext", bufs=4)
                        nc.scalar.activation(ext[:], pl[:], AF.Exp,
                                             bias=biask[:, i, c:c + 1])
                        egs = pcb.tile([128, 512], f32, tag="egs",
                                       name="egs", bufs=4)
                        nc.sync.dma_start(
                            egs[:], eg_d[i, 128 * c:128 * (c + 1), r0:r1])
                        exg = pcb.tile([128, 512], f32r, tag="exg",
                                       name="exg", bufs=3)
                        nc.gpsimd.tensor_tensor(exg[:], ext[:], egs[:],
                                                ALU.mult)
                        nc.tensor.matmul(pz[:], ezr[:, i, c, :], exg[:],
                                         start=(c == 0), stop=(c == 7))
                        nc.sync.dma_start(
                            expt_o[i, 128 * c:128 * (c + 1), r0:r1], ext[:])
                    rec = pcb.tile([1, 512], f32r, tag="rec", name="rec",
                                   bufs=2)
                    with nc.allow_low_precision(
                            reason="f32r recip feeds PE broadcast"):
                        nc.vector.reciprocal(rec[:], pz[64:65, :])
                    pb = psq.tile([64, 512], f32, tag="pb", name="pb")
                    nc.tensor.matmul(pb[:], ones64r[:], rec[:],
                                     start=True, stop=True)
                    rB = pcb.tile([64, 512], f32, tag="rB", name="rB",
                                  bufs=2)
                    nc.vector.tensor_copy(rB[:], pb[:])
                    z_sl = ztc[pb_lo:pb_lo + 64, i // 2, r0:r1]
                    nc.vector.scalar_tensor_tensor(
                        z_sl, pz[0:64, :], 1.0, rB[:], ALU.mult, ALU.mult)
                    nc.sync.dma_start(zt_o[i, :, r0:r1], z_sl)
        pclose(pcbcm)
        pclose(pccm)
        pclose(pycm)

        # =========================================================
        # FFN head
        # =========================================================
        p6cm, p6 = popen(name="p6", bufs=1, side="left")
        zcr = p6.tile([128, 2, NLOC], f32r, tag="zcr", name="zcr")
        nc.vector.tensor_copy(zcr[:].rearrange("p a b -> p (a b)"),
                              ztc[:].rearrange("p a b -> p (a b)"))
        pclose(pzcm)
        wlr = load_wr(p6, wlin_d, 2, H2, "wlr")
        embT = p6.tile([128, 2, NLOC], f32, tag="embT", name="embT")
        sE = glob.tile([128, 2, 4], f32, tag="sE", name="sE")
        with tc.tile_pool(name="ps6", bufs=4, space="PSUM") as ps6:
            for n in range(4):
                for m in range(2):
                    ps = ps6.tile([128, 512], f32, tag="mm", name="ps")
                    for k in range(2):
                        nc.tensor.matmul(
                            ps[:], wlr[:, k, 128 * m:128 * (m + 1)],
                            zcr[:, k, 512 * n:512 * (n + 1)],
                            start=(k == 0), stop=(k == 1))
                    nc.scalar.activation(
                        embT[:, m, 512 * n:512 * (n + 1)], ps[:], AF.Relu,
                        bias=blinv[:, m:m + 1], accum_out=sE[:, m, n:n + 1])
        stE = glob.tile([128, 4], f32, tag="stE", name="stE")
        nc.vector.tensor_reduce(stE[:, 0:2], sE[:], axis=AX.X, op=ALU.add)
        sumsq_pass(embT, 2, stE, 2)
        gstE = allreduce(3, stE[:], 4)
        aE, cE = bn_coeffs(3, gstE, 2, gb1g[:], gb1b[:])
        h1T = p6.tile([128, 2, NLOC], f32r, tag="h1T", name="h1T")
        for m in range(2):
            nc.scalar.activation(h1T[:, m, :], embT[:, m, :], AF.Identity,
                                 bias=cE[:, m:m + 1], scale=aE[:, m:m + 1])
        # fc1 -> gelu -> fc2
        wf1r = load_wr(p6, wf1_d, 2, FF, "wf1r")
        wf2r = load_wr(p6, wf2_d, 8, H2, "wf2r")
        h2p = p6.tile([128, 2, NLOC], f32, tag="h2p", name="h2p")
        sH = glob.tile([128, 2, 4], f32, tag="sH", name="sH")
        with tc.tile_pool(name="pg1", bufs=2) as pg1, \
             tc.tile_pool(name="ps7", bufs=4, space="PSUM") as ps7, \
             tc.tile_pool(name="ps8", bufs=2, space="PSUM") as ps8:
            for n in range(4):
                g1n = pg1.tile([128, 8, 512], f32r, tag="g1n", name="g1n")
                for m in range(8):
                    ps = ps7.tile([128, 512], f32, tag="mm", name="ps")
                    for k in range(2):
                        nc.tensor.matmul(
                            ps[:], wf1r[:, k, 128 * m:128 * (m + 1)],
                            h1T[:, k, 512 * n:512 * (n + 1)],
                            start=(k == 0), stop=(k == 1))
                    nc.scalar.activation(g1n[:, m, :], ps[:], AF.Gelu,
                                         bias=bf1v[:, m:m + 1])
                for m in range(2):
                    ps2b = ps8.tile([128, 512], f32, tag="mm2", name="ps2b")
                    for k in range(8):
                        nc.tensor.matmul(
                            ps2b[:], wf2r[:, k, 128 * m:128 * (m + 1)],
                            g1n[:, k, :], start=(k == 0), stop=(k == 7))
                    nc.scalar.activation(
                        h2p[:, m, 512 * n:512 * (n + 1)], ps2b[:],
                        AF.Identity, bias=bf2v[:, m:m + 1],
                        accum_out=sH[:, m, n:n + 1])
        stH = glob.tile([128, 4], f32, tag="stH", name="stH")
        nc.vector.tensor_reduce(stH[:, 0:2], sH[:], axis=AX.X, op=ALU.add)
        sumsq_pass(h2p, 2, stH, 2)
        gstH = allreduce(4, stH[:], 4)
        aH, cH = bn_coeffs(4, gstH, 2, gb2g[:], gb2b[:])
        # emb_out = h1 + bn2(h2p); rE = relu(emb_out)  (c folded into relu)
        eo = p6.tile([128, 2, NLOC], f32, tag="eo", name="eo")
        rE = p6.tile([128, 2, NLOC], f32r, tag="rE", name="rE")
        for m in range(2):
            nc.vector.scalar_tensor_tensor(
                eo[:, m, :], h2p[:, m, :], aH[:, m:m + 1], h1T[:, m, :],
                ALU.mult, ALU.add)
            nc.scalar.activation(rE[:, m, :], eo[:, m, :], AF.Relu,
                                 bias=cH[:, m:m + 1])
        # zmean / var heads
        wmr = load_wr(p6, wm_d, 2, ZD, "wmr")
        wvr = load_wr(p6, wv_d, 2, ZD, "wvr")
        zmT = p6.tile([128, NLOC], f32, tag="zmT", name="zmT")
        vrT = p6.tile([128, NLOC], f32, tag="vrT", name="vrT")
        with tc.tile_pool(name="ps9", bufs=4, space="PSUM") as ps9:
            for n in range(4):
                ps = ps9.tile([128, 512], f32, tag="mm", name="ps")
                for k in range(2):
                    nc.tensor.matmul(ps[:], wmr[:, k, :],
                                     rE[:, k, 512 * n:512 * (n + 1)],
                                     start=(k == 0), stop=(k == 1))
                nc.scalar.activation(zmT[:, 512 * n:512 * (n + 1)], ps[:],
                                     AF.Identity, bias=bmv[:])
                ps2c = ps9.tile([128, 512], f32, tag="mmv", name="ps2c")
                for k in range(2):
                    nc.tensor.matmul(ps2c[:], wvr[:, k, :],
                                     rE[:, k, 512 * n:512 * (n + 1)],
                                     start=(k == 0), stop=(k == 1))
                nc.scalar.activation(vrT[:, 512 * n:512 * (n + 1)], ps2c[:],
                                     AF.Exp, bias=bvv[:])
        nc.sync.dma_start(zm_o[:], zmT[:])
        nc.sync.dma_start(vr_o[:], vrT[:])
        pclose(p6cm)
        pclose(gcm)
        pclose(dcm)

    nc.compile()
    nc.m = get_hw_module(nc.m)
    return nc


def _prep_inputs(exp, gumbels, params):
    """Host-side shard + transpose prep. Returns in_maps (list of dicts)."""
    p = params
    f32 = np.float32

    def pk(v, w):  # pack a (F,) vector feature-major into (128, F//128)
        v = np.asarray(v, f32)
        return np.ascontiguousarray(v.reshape(-1, 128).T)

    expT = np.asarray(exp, f32).T                       # (2000, 16384)
    w0 = np.zeros((D_PAD, HID0), f32)
    w0[:D_IN] = np.asarray(p["enc0_W"], f32)
    w0 = w0.reshape(16, 128, HID0)

    eg = np.exp(np.asarray(gumbels, f32))               # (4, 16384, 1024)
    egT = np.ascontiguousarray(eg.transpose(0, 2, 1))   # (4, 1024, 16384)

    # (128, 2, K): chunk j rows 0-63 = 2*E_{2j}.T, rows 64-127 = 2*E_{2j+1}.T
    e2t = np.empty((128, 2, K), f32)
    for i, c in enumerate(p["cb"]):
        e2t[64 * (i % 2):64 * (i % 2) + 64, i // 2] = \
            2.0 * np.asarray(c["E"], f32).T
    ez = np.ones((NCB, 8, 128, CBD + 1), f32)
    for i, c in enumerate(p["cb"]):
        ez[i, :, :, :CBD] = np.asarray(c["E"], f32).reshape(8, 128, CBD)
    eet = np.stack([
        (np.sum(np.asarray(c["E"], f32) ** 2, axis=1) + LSHIFT)
        .reshape(8, 128).T.copy() for c in p["cb"]])    # (4, 128, 8)
    eet = np.ascontiguousarray(eet.transpose(1, 0, 2))  # (128, 4, 8)

    wcb = np.concatenate([np.asarray(c["W"], f32) for c in p["cb"]], axis=1)
    bcb = np.concatenate([np.asarray(c["b"], f32) for c in p["cb"]])
    gcb = np.concatenate([np.asarray(c["g"], f32) for c in p["cb"]])
    becb = np.concatenate([np.asarray(c["be"], f32) for c in p["cb"]])

    shared = {
        "w0": w0,
        "b0v": pk(p["enc0_b"], 8), "g0v": pk(p["enc0_g"], 8),
        "be0v": pk(p["enc0_be"], 8),
        "w1": np.asarray(p["enc1_W"], f32).reshape(8, 128, HID1),
        "b1v": pk(p["enc1_b"], 4), "g1v": pk(p["enc1_g"], 4),
        "be1v": pk(p["enc1_be"], 4),
        "w2": np.asarray(p["enc2_W"], f32).reshape(4, 128, EMB),
        "b2v": pk(p["enc2_b"], 4),
        "wcb": wcb.reshape(4, 128, NCB * CBD),
        "bcbv": pk(bcb, 2), "gcbv": pk(gcb, 2), "becbv": pk(becb, 2),
        "e2t": e2t, "ez": ez, "eet": eet,
        "wlin": np.asarray(p["lin_W"], f32).reshape(2, 128, H2),
        "blinv": pk(p["lin_b"], 2),
        "gb1g": pk(p["g1"], 2), "gb1b": pk(p["b1"], 2),
        "wfc1": np.asarray(p["fc1_W"], f32).reshape(2, 128, FF),
        "bfc1v": pk(p["fc1_b"], 8),
        "wfc2": np.asarray(p["fc2_W"], f32).reshape(8, 128, H2),
        "bfc2v": pk(p["fc2_b"], 2),
        "gb2g": pk(p["g2"], 2), "gb2b": pk(p["b2"], 2),
        "wmean": np.asarray(p["mean_W"], f32).reshape(2, 128, ZD),
        "bmeanv": pk(p["mean_b"], 1),
        "wvar": np.asarray(p["var_W"], f32).reshape(2, 128, ZD),
        "bvarv": pk(p["var_b"], 1),
    }

    in_maps = []
    for c in range(NCORES):
        cols = slice(c * NLOC, (c + 1) * NLOC)
        xc = np.zeros((D_PAD, NLOC), f32)
        xc[:D_IN] = expT[:, cols]
        # (2048k, 2048cols) -> (8 n-slabs, 16 kchunks, 128, 256)
        xin = np.ascontiguousarray(
            xc.reshape(16, 128, 8, 256).transpose(2, 0, 1, 3))
        m = dict(shared)
        m["xin"] = xin
        m["eg"] = np.ascontiguousarray(egT[:, :, cols])
        in_maps.append(m)
    return in_maps


def kernel(exp, gumbels, params):
    from concourse import bass_utils

    if "nc" not in _CACHE:
        _CACHE["nc"] = _build()
    nc = _CACHE["nc"]

    in_maps = _prep_inputs(exp, gumbels, params)
    res = bass_utils.run_bass_kernel_spmd(
        nc, in_maps, core_ids=list(range(NCORES)))
    _CACHE["last_result"] = res

    f32 = np.float32
    expt = np.concatenate([r["expt_out"] for r in res.results], axis=2)
    zt = np.concatenate([r["zt_out"] for r in res.results], axis=2)
    zm = np.concatenate([r["zmeant_out"] for r in res.results], axis=1)
    vr = np.concatenate([r["vart_out"] for r in res.results], axis=1)

    softs = []
    loss = np.float64(0.0)
    for i in range(NCB):
        e = expt[i]                                   # (K, N)
        s = e / e.sum(axis=0, keepdims=True)
        softs.append(np.ascontiguousarray(s.T, dtype=f32))
        idx = np.argmax(e, axis=0)                    # (N,)
        expected = np.asarray(params["cb"][i]["E"], f32)[idx]   # (N, CBD)
        z = zt[i].T                                   # (N, CBD)
        d = (z - expected).astype(np.float64)
        loss += np.mean(d * d)
    loss = np.float32((1.0 + BETA) * SCALER * loss)

    zmean = np.ascontiguousarray(zm.T, dtype=f32)
    variance = np.ascontiguousarray(vr.T, dtype=f32)
    variance = np.where(variance > 1e-6, variance, np.float32(1e-6))
    return (tuple(softs), zmean, variance, loss)


# revision 17
# speedup vs baseline: 1.1014x; 1.1014x over previous
"""Trainium2 Bass kernel for nn_Encoder_With_Codebook (vq_codebook).

Strategy: data-parallel over the 16384-cell batch across 8 NeuronCores, in a
"transposed world" layout (features on SBUF partitions, cells on the free dim)
so every linear layer chains without transposes and BatchNorm is per-partition.
BN batch statistics are all-reduced across cores in-kernel (5 tiny AllReduces).
Big matmuls run as float32r (full PE rate, ~1.5e-4 rel err). The per-codebook
softmax numerator exp(logits) is computed on device and emitted raw; the cheap
row-normalization, argmax/codebook-loss scalar, and final transposes are
finished on the host from device outputs.
"""
import sys

sys.path.insert(0, "/opt/trn_rl_repo")

import numpy as np

NCORES = 8
N = 16384
NLOC = N // NCORES          # 2048 rows per core
D_IN = 2000
D_PAD = 2048                # padded contraction for enc0
HID0, HID1, EMB = 1024, 512, 512
NCB, K, CBD = 4, 1024, 64
ZD = 128
H2 = 2 * ZD                 # 256
FF = 4 * H2                 # 1024
BETA, SCALER = 0.25, 1.0
EPS = 1e-5
LSHIFT = 40.0               # logit shift folded into ee to avoid fp32 overflow

_CACHE = {}


def _build():
    import concourse.bacc as bacc
    import concourse.tile as tile
    import concourse.mybir as mybir
    from concourse.bass_interp import get_hw_module

    dt = mybir.dt
    f32, f32r = dt.float32, dt.float32r
    AF = mybir.ActivationFunctionType
    ALU = mybir.AluOpType
    AX = mybir.AxisListType

    nc = bacc.Bacc("TRN2", target_bir_lowering=False, debug=False,
                   enable_asserts=False, num_devices=NCORES)

    def din(name, shape):
        return nc.dram_tensor(name, list(shape), f32, kind="ExternalInput").ap()

    def dout(name, shape):
        return nc.dram_tensor(name, list(shape), f32, kind="ExternalOutput").ap()

    # ---- inputs (per-core shards / replicated weights) ----
    xin_d = din("xin", (8, 16, 128, 256))         # exp.T padded, n-sliced
    eg_d = din("eg", (NCB, K, NLOC))              # exp(gumbels) transposed
    w0_d = din("w0", (16, 128, HID0))             # enc0_W padded (2048,1024)
    b0_d = din("b0v", (128, 8))
    g0_d = din("g0v", (128, 8))
    be0_d = din("be0v", (128, 8))
    w1_d = din("w1", (8, 128, HID1))
    b1_d = din("b1v", (128, 4))
    g1_d = din("g1v", (128, 4))
    be1_d = din("be1v", (128, 4))
    w2_d = din("w2", (4, 128, EMB))
    b2_d = din("b2v", (128, 4))
    wcb_d = din("wcb", (4, 128, NCB * CBD))       # concat cb W (512,256)
    bcb_d = din("bcbv", (128, 2))
    gcb_d = din("gcbv", (128, 2))
    becb_d = din("becbv", (128, 2))
    e2t_d = din("e2t", (128, 2, K))               # 2*E.T cb-interleaved
    ez_d = din("ez", (NCB, 8, 128, CBD + 1))      # [E | ones] k-chunked
    eet_d = din("eet", (128, NCB, 8))             # ||e||^2 + LSHIFT, k-parts
    wlin_d = din("wlin", (2, 128, H2))
    blin_d = din("blinv", (128, 2))
    gb1g_d = din("gb1g", (128, 2))
    gb1b_d = din("gb1b", (128, 2))
    wf1_d = din("wfc1", (2, 128, FF))
    bf1_d = din("bfc1v", (128, 8))
    wf2_d = din("wfc2", (8, 128, H2))
    bf2_d = din("bfc2v", (128, 2))
    gb2g_d = din("gb2g", (128, 2))
    gb2b_d = din("gb2b", (128, 2))
    wm_d = din("wmean", (2, 128, ZD))
    bm_d = din("bmeanv", (128, 1))
    wv_d = din("wvar", (2, 128, ZD))
    bv_d = din("bvarv", (128, 1))

    # ---- outputs ----
    expt_o = dout("expt_out", (NCB, K, NLOC))
    zt_o = dout("zt_out", (NCB, CBD, NLOC))
    zm_o = dout("zmeant_out", (ZD, NLOC))
    vr_o = dout("vart_out", (ZD, NLOC))

    RG = [list(range(NCORES))]
    INVN = 1.0 / float(N)

    with tile.TileContext(nc) as tc:
        open_pools = []

        def popen(**kw):
            cm = tc.tile_pool(**kw)
            pool = cm.__enter__()
            open_pools.append(cm)
            return cm, pool

        def pclose(cm):
            open_pools.remove(cm)
            cm.__exit__(None, None, None)

        gcm, glob = popen(name="glob", bufs=1, side="left")
        dcm, dpool = popen(name="dram", bufs=1, space="DRAM")

        # ---------- small helpers ----------
        def allreduce(idx, st_sbuf, width):
            ari = dpool.tile([128, width], f32, tag=f"ari{idx}",
                             name=f"ari{idx}")
            aro = dpool.tile([128, width], f32, tag=f"aro{idx}",
                             addr_space="Shared", name=f"aro{idx}")
            nc.sync.dma_start(ari[:], st_sbuf)
            nc.gpsimd.collective_compute(
                "AllReduce", ALU.add, replica_groups=RG,
                ins=[ari[:]], outs=[aro[:]])
            gst = glob.tile([128, width], f32, tag=f"gst{idx}",
                            name=f"gst{idx}")
            nc.sync.dma_start(gst[:], aro[:])
            return gst

        def bn_coeffs(idx, gst, w, g_ap, be_ap):
            va = glob.tile([128, w], f32, tag=f"va{idx}", name=f"va{idx}")
            sd = glob.tile([128, w], f32, tag=f"sd{idx}", name=f"sd{idx}")
            aa = glob.tile([128, w], f32, tag=f"aa{idx}", name=f"aa{idx}")
            cc = glob.tile([128, w], f32, tag=f"cc{idx}", name=f"cc{idx}")
            me = glob.tile([128, 2 * w], f32, tag=f"me{idx}", name=f"me{idx}")
            nc.vector.tensor_scalar_mul(me[:], gst[:, 0:2 * w], INVN)
            mu = me[:, 0:w]
            nc.vector.tensor_mul(sd[:], mu, mu)
            nc.vector.tensor_sub(va[:], me[:, w:2 * w], sd[:])
            nc.scalar.activation(sd[:], va[:], AF.Sqrt, bias=epsv[:, 0:1])
            nc.vector.reciprocal(aa[:], sd[:])
            nc.vector.tensor_mul(aa[:], aa[:], g_ap)
            nc.vector.tensor_mul(cc[:], mu, aa[:])
            nc.vector.tensor_sub(cc[:], be_ap, cc[:])
            return aa, cc

        def load_small(ap_d, shape, tag):
            t = glob.tile(list(shape), f32, tag=tag, name=tag)
            nc.sync.dma_start(t[:], ap_d[:])
            return t

        def load_wr(pool, dram3, kchunks, mwidth, tag):
            """Load (kchunks,128,mwidth) weights, convert to f32r chunkwise."""
            wr = pool.tile([128, kchunks, mwidth], f32r, tag=tag, name=tag)
            for k in range(kchunks):
                ws = glob.tile([128, 1024], f32, tag="wstage", name="ws",
                               bufs=2)
                nc.sync.dma_start(ws[:, 0:mwidth], dram3[k])
                nc.vector.tensor_copy(wr[:, k, :], ws[:, 0:mwidth])
            return wr

        epsv = glob.tile([128, 1], f32, tag="epsv", name="epsv")
        nc.vector.memset(epsv[:], EPS)

        b0v = load_small(b0_d, (128, 8), "b0v")
        g0v = load_small(g0_d, (128, 8), "g0v")
        be0v = load_small(be0_d, (128, 8), "be0v")
        b1v = load_small(b1_d, (128, 4), "b1v")
        g1v = load_small(g1_d, (128, 4), "g1v")
        be1v = load_small(be1_d, (128, 4), "be1v")
        b2v = load_small(b2_d, (128, 4), "b2v")
        bcbv = load_small(bcb_d, (128, 2), "bcbv")
        gcbv = load_small(gcb_d, (128, 2), "gcbv")
        becbv = load_small(becb_d, (128, 2), "becbv")
        blinv = load_small(blin_d, (128, 2), "blinv")
        gb1g = load_small(gb1g_d, (128, 2), "gb1g")
        gb1b = load_small(gb1b_d, (128, 2), "gb1b")
        bf1v = load_small(bf1_d, (128, 8), "bf1v")
        bf2v = load_small(bf2_d, (128, 2), "bf2v")
        gb2g = load_small(gb2g_d, (128, 2), "gb2g")
        gb2b = load_small(gb2b_d, (128, 2), "gb2b")
        bmv = load_small(bm_d, (128, 1), "bmv")
        bvv = load_small(bv_d, (128, 1), "bvv")
        eets = load_small(eet_d, (128, NCB * 8), "eets")

        # dummy collective to absorb first-AllReduce staging cost; overlaps
        # with enc0 compute
        allreduce(9, epsv[:], 1)

        # =========================================================
        # enc0: x0 = exp @ W0 + b0   (k=16 chunks, m=8, n=8 x 256)
        # =========================================================
        p1cm, p1 = popen(name="p1", bufs=1, side="left")     # w0r + xin/xr staging
        p2cm, p2 = popen(name="p2", bufs=1, side="right")     # x0_pre
        w0r = load_wr(p1, w0_d, 16, HID0, "w0r")
        x0_pre = p2.tile([128, 8, NLOC], f32, tag="x0pre", name="x0_pre")
        s0acc = glob.tile([128, 8, 8], f32, tag="s0acc", name="s0acc")
        q0acc = glob.tile([128, 8, 8], f32, tag="q0acc", name="q0acc")
        with tc.tile_pool(name="ps0", bufs=4, space="PSUM") as ps0, \
             tc.tile_pool(name="sq0", bufs=3, side="left") as sq0:
            for n in range(8):
                xr = p1.tile([128, 16, 256], f32r, tag="xr", name="xr",
                             bufs=2)
                for kq in range(4):
                    xin_s = p1.tile([128, 4, 256], f32, tag="xins",
                                    name="xin_s", bufs=2)
                    nc.sync.dma_start(xin_s[:], xin_d[n, 4 * kq:4 * kq + 4]
                                      .rearrange("k p w -> p k w"))
                    nc.vector.tensor_copy(xr[:, 4 * kq:4 * kq + 4, :],
                                          xin_s[:])
                for m in range(8):
                    ps = ps0.tile([128, 256], f32, tag="mm", name="ps")
                    for k in range(16):
                        nc.tensor.matmul(
                            ps[:], w0r[:, k, 128 * m:128 * (m + 1)],
                            xr[:, k, :], start=(k == 0), stop=(k == 15))
                    xsl = x0_pre[:, m, 256 * n:256 * (n + 1)]
                    nc.scalar.activation(
                        xsl, ps[:],
                        AF.Identity, bias=b0v[:, m:m + 1],
                        accum_out=s0acc[:, m, n:n + 1])
                    sc = sq0.tile([128, 256], f32, tag="sc", name="sc")
                    nc.vector.scalar_tensor_tensor(
                        sc[:], xsl, 0.0, xsl, ALU.add, ALU.mult,
                        accum_out=q0acc[:, m, n:n + 1])
        pclose(p1cm)
        st0 = glob.tile([128, 16], f32, tag="st0", name="st0")
        nc.vector.tensor_reduce(st0[:, 0:8], s0acc[:], axis=AX.X, op=ALU.add)
        nc.vector.tensor_reduce(st0[:, 8:16], q0acc[:], axis=AX.X, op=ALU.add)
        gst0 = allreduce(0, st0[:], 16)
        a0, c0 = bn_coeffs(0, gst0, 8, g0v[:], be0v[:])
        px0cm, px0 = popen(name="px0", bufs=1, side="left")
        x0r = px0.tile([128, 8, NLOC], f32r, tag="x0r", name="x0r")
        for m in range(8):
            nc.scalar.activation(x0r[:, m, :], x0_pre[:, m, :], AF.Relu,
                                 bias=c0[:, m:m + 1], scale=a0[:, m:m + 1])
        pclose(p2cm)

        # =========================================================
        # enc1: x1 = relu(bn(x0)) @ W1 + b1   (k=8, m=4, n=4 x 512)
        # =========================================================
        p3cm, p3 = popen(name="p3", bufs=1, side="right")
        w1r = load_wr(p3, w1_d, 8, HID1, "w1r")
        x1_pre = p3.tile([128, 4, NLOC], f32, tag="x1pre", name="x1_pre")
        s1acc = glob.tile([128, 4, 4], f32, tag="s1acc", name="s1acc")
        q1acc = glob.tile([128, 4, 4], f32, tag="q1acc", name="q1acc")
        with tc.tile_pool(name="ps1", bufs=4, space="PSUM") as ps1, \
             tc.tile_pool(name="sq1", bufs=3, side="left") as sq1:
            for n in range(4):
                for m in range(4):
                    ps = ps1.tile([128, 512], f32, tag="mm", name="ps")
                    for k in range(8):
                        nc.tensor.matmul(
                            ps[:], w1r[:, k, 128 * m:128 * (m + 1)],
                            x0r[:, k, 512 * n:512 * (n + 1)],
                            start=(k == 0), stop=(k == 7))
                    xsl = x1_pre[:, m, 512 * n:512 * (n + 1)]
                    nc.scalar.activation(
                        xsl, ps[:],
                        AF.Identity, bias=b1v[:, m:m + 1],
                        accum_out=s1acc[:, m, n:n + 1])
                    sc = sq1.tile([128, 512], f32, tag="sc", name="sc")
                    nc.vector.scalar_tensor_tensor(
                        sc[:], xsl, 0.0, xsl, ALU.add, ALU.mult,
                        accum_out=q1acc[:, m, n:n + 1])
        pclose(px0cm)
        st1 = glob.tile([128, 8], f32, tag="st1", name="st1")
        nc.vector.tensor_reduce(st1[:, 0:4], s1acc[:], axis=AX.X, op=ALU.add)
        nc.vector.tensor_reduce(st1[:, 4:8], q1acc[:], axis=AX.X, op=ALU.add)
        gst1 = allreduce(1, st1[:], 8)
        a1, c1 = bn_coeffs(1, gst1, 4, g1v[:], be1v[:])
        px1cm, px1 = popen(name="px1", bufs=1, side="left")
        x1r = px1.tile([128, 4, NLOC], f32r, tag="x1r", name="x1r")
        for m in range(4):
            nc.scalar.activation(x1r[:, m, :], x1_pre[:, m, :], AF.Relu,
                                 bias=c1[:, m:m + 1], scale=a1[:, m:m + 1])
        pclose(p3cm)

        # =========================================================
        # enc2: xe = relu(bn(x1)) @ W2 + b2 ; cbproj: y = xe@Wcat + bcat
        # =========================================================
        p4cm, p4 = popen(name="p4", bufs=1, side="right")
        w2r = load_wr(p4, w2_d, 4, EMB, "w2r")
        xe = p4.tile([128, 4, NLOC], f32r, tag="xe", name="xe")
        with tc.tile_pool(name="ps2", bufs=4, space="PSUM") as ps2:
            for n in range(4):
                for m in range(4):
                    ps = ps2.tile([128, 512], f32, tag="mm", name="ps")
                    for k in range(4):
                        nc.tensor.matmul(
                            ps[:], w2r[:, k, 128 * m:128 * (m + 1)],
                            x1r[:, k, 512 * n:512 * (n + 1)],
                            start=(k == 0), stop=(k == 3))
                    nc.scalar.activation(
                        xe[:, m, 512 * n:512 * (n + 1)], ps[:],
                        AF.Identity, bias=b2v[:, m:m + 1])
        pclose(px1cm)
        pycm, py = popen(name="py", bufs=1, side="left")
        y_pre = py.tile([128, 2, NLOC], f32r, tag="ypre", name="y_pre")
        sYacc = glob.tile([128, 2, 4], f32, tag="syacc", name="sYacc")
        qYacc = glob.tile([128, 2, 4], f32, tag="qyacc", name="qYacc")
        wcr = load_wr(p4, wcb_d, 4, NCB * CBD, "wcr")
        with tc.tile_pool(name="ps3", bufs=4, space="PSUM") as ps3, \
             tc.tile_pool(name="sqY", bufs=3, side="left") as sqY:
            for n in range(4):
                for m in range(2):
                    ps = ps3.tile([128, 512], f32, tag="mm", name="ps")
                    for k in range(4):
                        nc.tensor.matmul(
                            ps[:], wcr[:, k, 128 * m:128 * (m + 1)],
                            xe[:, k, 512 * n:512 * (n + 1)],
                            start=(k == 0), stop=(k == 3))
                    ysl = y_pre[:, m, 512 * n:512 * (n + 1)]
                    nc.scalar.activation(
                        ysl, ps[:],
                        AF.Identity, bias=bcbv[:, m:m + 1],
                        accum_out=sYacc[:, m, n:n + 1])
                    sc = sqY.tile([128, 512], f32, tag="sc", name="sc")
                    nc.vector.scalar_tensor_tensor(
                        sc[:], ysl, 0.0, ysl, ALU.add, ALU.mult,
                        accum_out=qYacc[:, m, n:n + 1])
        pclose(p4cm)
        stY = glob.tile([128, 4], f32, tag="stY", name="stY")
        nc.vector.tensor_reduce(stY[:, 0:2], sYacc[:], axis=AX.X, op=ALU.add)
        nc.vector.tensor_reduce(stY[:, 2:4], qYacc[:], axis=AX.X, op=ALU.add)
        gstY = allreduce(2, stY[:], 4)
        aY, cY = bn_coeffs(2, gstY, 2, gcbv[:], becbv[:])

        # ---- codebook constants: E2a = a*(2E.T) f32r, biask, Ez f32r ----
        pccm, pcc = popen(name="pcc", bufs=1, side="left")    # e2a, ezr (persist cb phase)
        e2a = pcc.tile([128, 2, K], f32r, tag="e2a", name="e2a")
        ezr = pcc.tile([128, NCB, 8, CBD + 1], f32r, tag="ezr", name="ezr")
        biask = glob.tile([128, NCB, 8], f32, tag="biask", name="biask")
        c2t = glob.tile([128, 2], f32, tag="c2t", name="c2t")
        ones64 = glob.tile([1, 64], f32, tag="ones64", name="ones64")
        nc.vector.memset(ones64[:], 1.0)
        ones64r = glob.tile([1, 64], f32r, tag="ones64r", name="ones64r")
        nc.vector.tensor_copy(ones64r[:], ones64[:])
        pstcm, pst = popen(name="pst", bufs=1, side="left")   # f32 staging, closed early
        e2s = pst.tile([128, 2, K], f32, tag="e2s", name="e2s")
        nc.sync.dma_start(e2s[:], e2t_d[:])
        ezs = pst.tile([128, NCB, 8, CBD + 1], f32, tag="ezs", name="ezs")
        nc.sync.dma_start(ezs[:], ez_d.rearrange("i c p w -> p i c w"))
        nc.vector.tensor_copy(ezr[:], ezs[:])
        # e2s already carries the 2x factor, so bias = (2E).c - ee uses c as-is
        nc.vector.tensor_copy(c2t[:], cY[:])
        for j in range(2):
            nc.vector.tensor_scalar(e2a[:, j, :], e2s[:, j, :],
                                    aY[:, j:j + 1], None, ALU.mult)
        with tc.tile_pool(name="psb0", bufs=2, space="PSUM") as psb0:
            for i in range(NCB):
                pb_lo = 64 * (i % 2)
                for c in range(8):
                    psb = psb0.tile([128, 1], f32, tag="psb", name="psb")
                    nc.tensor.matmul(
                        psb[:],
                        e2s[pb_lo:pb_lo + 64, i // 2, 128 * c:128 * (c + 1)],
                        c2t[pb_lo:pb_lo + 64, i // 2:i // 2 + 1],
                        start=True, stop=True)
                    nc.vector.tensor_sub(
                        biask[:, i, c:c + 1], psb[:],
                        eets[:, 8 * i + c:8 * i + c + 1])
        pclose(pstcm)

        # =========================================================
        # codebook phase: logits -> exp -> (x eg) -> z, per (i, rt)
        # =========================================================
        pzcm, pzt = popen(name="pzt", bufs=1, side="right")
        ztc = pzt.tile([128, 2, NLOC], f32, tag="ztc", name="ztc")
        pcbcm, pcb = popen(name="pcb", bufs=1, side="left")
        with tc.tile_pool(name="psl", bufs=2, space="PSUM") as psl, \
             tc.tile_pool(name="psz", bufs=1, space="PSUM") as psz, \
             tc.tile_pool(name="psq", bufs=1, space="PSUM") as psq:
            for i in range(NCB):
                pb_lo = 64 * (i % 2)
                # 4 z/sumexp accumulators live across the c loop (one/rowtile)
                pzs = [psz.tile([65, 512], f32, tag=f"pz{rt}",
                                name=f"pz{rt}") for rt in range(4)]
                for c in range(8):
                    egs = pcb.tile([128, NLOC], f32, tag="egs",
                                   name="egs", bufs=2)
                    nc.sync.dma_start(egs[:],
                                      eg_d[i, 128 * c:128 * (c + 1), :])
                    ext = pcb.tile([128, NLOC], f32, tag="ext",
                                   name="ext", bufs=2)
                    for rt in range(4):
                        r0, r1 = 512 * rt, 512 * (rt + 1)
                        pl = psl.tile([128, 512], f32, tag="pl", name="pl")
                        nc.tensor.matmul(
                            pl[:],
                            e2a[pb_lo:pb_lo + 64, i // 2,
                                128 * c:128 * (c + 1)],
                            y_pre[pb_lo:pb_lo + 64, i // 2, r0:r1],
                            start=True, stop=True)
                        nc.scalar.activation(ext[:, r0:r1], pl[:], AF.Exp,
                                             bias=biask[:, i, c:c + 1])
                        exg = pcb.tile([128, 512], f32r, tag="exg",
                                       name="exg", bufs=4)
                        nc.gpsimd.tensor_tensor(exg[:], ext[:, r0:r1],
                                                egs[:, r0:r1], ALU.mult)
                        nc.tensor.matmul(pzs[rt][:], ezr[:, i, c, :], exg[:],
                                         start=(c == 0), stop=(c == 7))
                    nc.sync.dma_start(
                        expt_o[i, 128 * c:128 * (c + 1), :], ext[:])
                for rt in range(4):
                    r0, r1 = 512 * rt, 512 * (rt + 1)
                    pz = pzs[rt]
                    rec = pcb.tile([1, 512], f32r, tag="rec", name="rec",
                                   bufs=2)
                    with nc.allow_low_precision(
                            reason="f32r recip feeds PE broadcast"):
                        nc.vector.reciprocal(rec[:], pz[64:65, :])
                    pb = psq.tile([64, 512], f32, tag="pb", name="pb",
                                  bufs=2)
                    nc.tensor.matmul(pb[:], ones64r[:], rec[:],
                                     start=True, stop=True)
                    rB = pcb.tile([64, 512], f32, tag="rB", name="rB",
                                  bufs=2)
                    nc.vector.tensor_copy(rB[:], pb[:])
                    z_sl = ztc[pb_lo:pb_lo + 64, i // 2, r0:r1]
                    nc.vector.scalar_tensor_tensor(
                        z_sl, pz[0:64, :], 1.0, rB[:], ALU.mult, ALU.mult)
                    nc.sync.dma_start(zt_o[i, :, r0:r1], z_sl)
        pclose(pcbcm)
        pclose(pccm)
        pclose(pycm)

        # =========================================================
        # FFN head
        # =========================================================
        p6cm, p6 = popen(name="p6", bufs=1, side="left")
        zcr = p6.tile([128, 2, NLOC], f32r, tag="zcr", name="zcr")
        nc.vector.tensor_copy(zcr[:].rearrange("p a b -> p (a b)"),
                              ztc[:].rearrange("p a b -> p (a b)"))
        pclose(pzcm)
        wlr = load_wr(p6, wlin_d, 2, H2, "wlr")
        embT = p6.tile([128, 2, NLOC], f32, tag="embT", name="embT")
        sE = glob.tile([128, 2, 4], f32, tag="sE", name="sE")
        qE = glob.tile([128, 2, 4], f32, tag="qE", name="qE")
        with tc.tile_pool(name="ps6", bufs=4, space="PSUM") as ps6, \
             tc.tile_pool(name="sqE", bufs=3, side="left") as sqE:
            for n in range(4):
                for m in range(2):
                    ps = ps6.tile([128, 512], f32, tag="mm", name="ps")
                    for k in range(2):
                        nc.tensor.matmul(
                            ps[:], wlr[:, k, 128 * m:128 * (m + 1)],
                            zcr[:, k, 512 * n:512 * (n + 1)],
                            start=(k == 0), stop=(k == 1))
                    esl = embT[:, m, 512 * n:512 * (n + 1)]
                    nc.scalar.activation(
                        esl, ps[:], AF.Relu,
                        bias=blinv[:, m:m + 1], accum_out=sE[:, m, n:n + 1])
                    sc = sqE.tile([128, 512], f32, tag="sc", name="sc")
                    nc.vector.scalar_tensor_tensor(
                        sc[:], esl, 0.0, esl, ALU.add, ALU.mult,
                        accum_out=qE[:, m, n:n + 1])
        stE = glob.tile([128, 4], f32, tag="stE", name="stE")
        nc.vector.tensor_reduce(stE[:, 0:2], sE[:], axis=AX.X, op=ALU.add)
        nc.vector.tensor_reduce(stE[:, 2:4], qE[:], axis=AX.X, op=ALU.add)
        gstE = allreduce(3, stE[:], 4)
        aE, cE = bn_coeffs(3, gstE, 2, gb1g[:], gb1b[:])
        h1T = p6.tile([128, 2, NLOC], f32r, tag="h1T", name="h1T")
        for m in range(2):
            nc.scalar.activation(h1T[:, m, :], embT[:, m, :], AF.Identity,
                                 bias=cE[:, m:m + 1], scale=aE[:, m:m + 1])
        # fc1 -> gelu -> fc2
        wf1r = load_wr(p6, wf1_d, 2, FF, "wf1r")
        wf2r = load_wr(p6, wf2_d, 8, H2, "wf2r")
        h2p = p6.tile([128, 2, NLOC], f32, tag="h2p", name="h2p")
        sH = glob.tile([128, 2, 4], f32, tag="sH", name="sH")
        qH = glob.tile([128, 2, 4], f32, tag="qH", name="qH")
        with tc.tile_pool(name="pg1", bufs=2) as pg1, \
             tc.tile_pool(name="sqH", bufs=3, side="left") as sqH, \
             tc.tile_pool(name="ps7", bufs=4, space="PSUM") as ps7, \
             tc.tile_pool(name="ps8", bufs=2, space="PSUM") as ps8:
            for n in range(4):
                g1n = pg1.tile([128, 8, 512], f32r, tag="g1n", name="g1n")
                for m in range(8):
                    ps = ps7.tile([128, 512], f32, tag="mm", name="ps")
                    for k in range(2):
                        nc.tensor.matmul(
                            ps[:], wf1r[:, k, 128 * m:128 * (m + 1)],
                            h1T[:, k, 512 * n:512 * (n + 1)],
                            start=(k == 0), stop=(k == 1))
                    nc.scalar.activation(g1n[:, m, :], ps[:], AF.Gelu,
                                         bias=bf1v[:, m:m + 1])
                for m in range(2):
                    ps2b = ps8.tile([128, 512], f32, tag="mm2", name="ps2b")
                    for k in range(8):
                        nc.tensor.matmul(
                            ps2b[:], wf2r[:, k, 128 * m:128 * (m + 1)],
                            g1n[:, k, :], start=(k == 0), stop=(k == 7))
                    hsl = h2p[:, m, 512 * n:512 * (n + 1)]
                    nc.scalar.activation(
                        hsl, ps2b[:],
                        AF.Identity, bias=bf2v[:, m:m + 1],
                        accum_out=sH[:, m, n:n + 1])
                    sc = sqH.tile([128, 512], f32, tag="sc", name="sc")
                    nc.vector.scalar_tensor_tensor(
                        sc[:], hsl, 0.0, hsl, ALU.add, ALU.mult,
                        accum_out=qH[:, m, n:n + 1])
        stH = glob.tile([128, 4], f32, tag="stH", name="stH")
        nc.vector.tensor_reduce(stH[:, 0:2], sH[:], axis=AX.X, op=ALU.add)
        nc.vector.tensor_reduce(stH[:, 2:4], qH[:], axis=AX.X, op=ALU.add)
        gstH = allreduce(4, stH[:], 4)
        aH, cH = bn_coeffs(4, gstH, 2, gb2g[:], gb2b[:])
        # emb_out = h1 + bn2(h2p); rE = relu(emb_out)  (c folded into relu)
        eo = p6.tile([128, 2, NLOC], f32, tag="eo", name="eo")
        rE = p6.tile([128, 2, NLOC], f32r, tag="rE", name="rE")
        for m in range(2):
            nc.vector.scalar_tensor_tensor(
                eo[:, m, :], h2p[:, m, :], aH[:, m:m + 1], h1T[:, m, :],
                ALU.mult, ALU.add)
            nc.scalar.activation(rE[:, m, :], eo[:, m, :], AF.Relu,
                                 bias=cH[:, m:m + 1])
        # zmean / var heads
        wmr = load_wr(p6, wm_d, 2, ZD, "wmr")
        wvr = load_wr(p6, wv_d, 2, ZD, "wvr")
        zmT = p6.tile([128, NLOC], f32, tag="zmT", name="zmT")
        vrT = p6.tile([128, NLOC], f32, tag="vrT", name="vrT")
        with tc.tile_pool(name="ps9", bufs=4, space="PSUM") as ps9:
            for n in range(4):
                ps = ps9.tile([128, 512], f32, tag="mm", name="ps")
                for k in range(2):
                    nc.tensor.matmul(ps[:], wmr[:, k, :],
                                     rE[:, k, 512 * n:512 * (n + 1)],
                                     start=(k == 0), stop=(k == 1))
                nc.scalar.activation(zmT[:, 512 * n:512 * (n + 1)], ps[:],
                                     AF.Identity, bias=bmv[:])
                ps2c = ps9.tile([128, 512], f32, tag="mmv", name="ps2c")
                for k in range(2):
                    nc.tensor.matmul(ps2c[:], wvr[:, k, :],
                                     rE[:, k, 512 * n:512 * (n + 1)],
                                     start=(k == 0), stop=(k == 1))
                nc.scalar.activation(vrT[:, 512 * n:512 * (n + 1)], ps2c[:],
                                     AF.Exp, bias=bvv[:])
        nc.sync.dma_start(zm_o[:], zmT[:])
        nc.sync.dma_start(vr_o[:], vrT[:])
        pclose(p6cm)
        pclose(gcm)
        pclose(dcm)

    nc.compile()
    nc.m = get_hw_module(nc.m)
    return nc


def _prep_inputs(exp, gumbels, params):
    """Host-side shard + transpose prep. Returns in_maps (list of dicts)."""
    p = params
    f32 = np.float32

    def pk(v, w):  # pack a (F,) vector feature-major into (128, F//128)
        v = np.asarray(v, f32)
        return np.ascontiguousarray(v.reshape(-1, 128).T)

    expT = np.asarray(exp, f32).T                       # (2000, 16384)
    w0 = np.zeros((D_PAD, HID0), f32)
    w0[:D_IN] = np.asarray(p["enc0_W"], f32)
    w0 = w0.reshape(16, 128, HID0)

    eg = np.exp(np.asarray(gumbels, f32))               # (4, 16384, 1024)
    egT = np.ascontiguousarray(eg.transpose(0, 2, 1))   # (4, 1024, 16384)

    # (128, 2, K): chunk j rows 0-63 = 2*E_{2j}.T, rows 64-127 = 2*E_{2j+1}.T
    e2t = np.empty((128, 2, K), f32)
    for i, c in enumerate(p["cb"]):
        e2t[64 * (i % 2):64 * (i % 2) + 64, i // 2] = \
            2.0 * np.asarray(c["E"], f32).T
    ez = np.ones((NCB, 8, 128, CBD + 1), f32)
    for i, c in enumerate(p["cb"]):
        ez[i, :, :, :CBD] = np.asarray(c["E"], f32).reshape(8, 128, CBD)
    eet = np.stack([
        (np.sum(np.asarray(c["E"], f32) ** 2, axis=1) + LSHIFT)
        .reshape(8, 128).T.copy() for c in p["cb"]])    # (4, 128, 8)
    eet = np.ascontiguousarray(eet.transpose(1, 0, 2))  # (128, 4, 8)

    wcb = np.concatenate([np.asarray(c["W"], f32) for c in p["cb"]], axis=1)
    bcb = np.concatenate([np.asarray(c["b"], f32) for c in p["cb"]])
    gcb = np.concatenate([np.asarray(c["g"], f32) for c in p["cb"]])
    becb = np.concatenate([np.asarray(c["be"], f32) for c in p["cb"]])

    shared = {
        "w0": w0,
        "b0v": pk(p["enc0_b"], 8), "g0v": pk(p["enc0_g"], 8),
        "be0v": pk(p["enc0_be"], 8),
        "w1": np.asarray(p["enc1_W"], f32).reshape(8, 128, HID1),
        "b1v": pk(p["enc1_b"], 4), "g1v": pk(p["enc1_g"], 4),
        "be1v": pk(p["enc1_be"], 4),
        "w2": np.asarray(p["enc2_W"], f32).reshape(4, 128, EMB),
        "b2v": pk(p["enc2_b"], 4),
        "wcb": wcb.reshape(4, 128, NCB * CBD),
        "bcbv": pk(bcb, 2), "gcbv": pk(gcb, 2), "becbv": pk(becb, 2),
        "e2t": e2t, "ez": ez, "eet": eet,
        "wlin": np.asarray(p["lin_W"], f32).reshape(2, 128, H2),
        "blinv": pk(p["lin_b"], 2),
        "gb1g": pk(p["g1"], 2), "gb1b": pk(p["b1"], 2),
        "wfc1": np.asarray(p["fc1_W"], f32).reshape(2, 128, FF),
        "bfc1v": pk(p["fc1_b"], 8),
        "wfc2": np.asarray(p["fc2_W"], f32).reshape(8, 128, H2),
        "bfc2v": pk(p["fc2_b"], 2),
        "gb2g": pk(p["g2"], 2), "gb2b": pk(p["b2"], 2),
        "wmean": np.asarray(p["mean_W"], f32).reshape(2, 128, ZD),
        "bmeanv": pk(p["mean_b"], 1),
        "wvar": np.asarray(p["var_W"], f32).reshape(2, 128, ZD),
        "bvarv": pk(p["var_b"], 1),
    }

    in_maps = []
    for c in range(NCORES):
        cols = slice(c * NLOC, (c + 1) * NLOC)
        xc = np.zeros((D_PAD, NLOC), f32)
        xc[:D_IN] = expT[:, cols]
        # (2048k, 2048cols) -> (8 n-slabs, 16 kchunks, 128, 256)
        xin = np.ascontiguousarray(
            xc.reshape(16, 128, 8, 256).transpose(2, 0, 1, 3))
        m = dict(shared)
        m["xin"] = xin
        m["eg"] = np.ascontiguousarray(egT[:, :, cols])
        in_maps.append(m)
    return in_maps


def kernel(exp, gumbels, params):
    from concourse import bass_utils

    if "nc" not in _CACHE:
        _CACHE["nc"] = _build()
    nc = _CACHE["nc"]

    in_maps = _prep_inputs(exp, gumbels, params)
    res = bass_utils.run_bass_kernel_spmd(
        nc, in_maps, core_ids=list(range(NCORES)))
    _CACHE["last_result"] = res

    f32 = np.float32
    expt = np.concatenate([r["expt_out"] for r in res.results], axis=2)
    zt = np.concatenate([r["zt_out"] for r in res.results], axis=2)
    zm = np.concatenate([r["zmeant_out"] for r in res.results], axis=1)
    vr = np.concatenate([r["vart_out"] for r in res.results], axis=1)

    softs = []
    loss = np.float64(0.0)
    for i in range(NCB):
        e = expt[i]                                   # (K, N)
        s = e / e.sum(axis=0, keepdims=True)
        softs.append(np.ascontiguousarray(s.T, dtype=f32))
        idx = np.argmax(e, axis=0)                    # (N,)
        expected = np.asarray(params["cb"][i]["E"], f32)[idx]   # (N, CBD)
        z = zt[i].T                                   # (N, CBD)
        d = (z - expected).astype(np.float64)
        loss += np.mean(d * d)
    loss = np.float32((1.0 + BETA) * SCALER * loss)

    zmean = np.ascontiguousarray(zm.T, dtype=f32)
    variance = np.ascontiguousarray(vr.T, dtype=f32)
    variance = np.where(variance > 1e-6, variance, np.float32(1e-6))
    return (tuple(softs), zmean, variance, loss)
